# revision 4
# baseline (speedup 1.0000x reference)
# Trainium2 Bass kernel for nn_Attention: out = softmax(x @ (y@W + b) + mask*-1e9) @ x
# Sharding: data-parallel over batch, 1 batch element per NeuronCore (8 cores).
#
# Per-core math (S = D = 1024):
#   logits = x @ (y @ W) + rowsum(x) (x) b + mask * -1e9      [reassociated: (x@y)@W]
#   out    = softmax(logits) @ x
#
# Precision strategy: the logit chain runs as hi/lo-split fp16 matmuls (3 passes
# per matmul, fp16 products are exact in fp32 PSUM), giving near-fp32 logits at
# bf16-tier speed. The output matmul runs as a single fp16 pass. Inputs are
# pre-scaled (x,y by 16, W by 1024) so the fp16 "lo" residuals stay in normal
# fp16 range; the resulting 2^18 logit scale is folded into the exp affine.
import sys

import numpy as np

for _p in ("/opt/trn_rl_repo",):
    if _p not in sys.path:
        sys.path.insert(0, _p)

import concourse.bass as bass
from concourse import bacc
import concourse.mybir as mybir
import concourse.tile as tile
from concourse.bass_utils import run_bass_kernel_spmd

F32 = mybir.dt.float32
F16 = mybir.dt.float16

B = 8
P = 128
FD = 512  # matmul moving free dim (one fp32 PSUM bank)

SX = 16.0  # x / y pre-scale
SW = 1024.0  # W pre-scale
SLOG = SX * SX * SW  # net logit scale = 2**18
MASKC = -1.0e9 * SLOG

ALU = mybir.AluOpType
ACTF = mybir.ActivationFunctionType
AXIS = mybir.AxisListType


def build_nc(n=1024):
    """Build the per-core Bass program (SPMD: same program on all 8 cores)."""
    NT = n // P  # 128-tiles per dim
    NH = n // FD  # 512-halves per dim

    nc = bacc.Bacc("TRN2", target_bir_lowering=False, debug=False)
    x_d = nc.dram_tensor("x", [n, n], F32, kind="ExternalInput")
    y_d = nc.dram_tensor("y", [n, n], F32, kind="ExternalInput")
    mask_d = nc.dram_tensor("mask", [n, n], F32, kind="ExternalInput")
    w_d = nc.dram_tensor("W", [n, n], F32, kind="ExternalInput")
    b_d = nc.dram_tensor("bvec", [1, n], F32, kind="ExternalInput")
    out_d = nc.dram_tensor("out", [n, n], F32, kind="ExternalOutput")

    with tile.TileContext(nc) as tc:
        import contextlib

        ctx = contextlib.ExitStack()
        with ctx:
            persist = ctx.enter_context(tc.tile_pool(name="persist", bufs=1))
            ld = ctx.enter_context(tc.tile_pool(name="ld", bufs=3))
            epi = ctx.enter_context(tc.tile_pool(name="epi", bufs=2))
            small = ctx.enter_context(tc.tile_pool(name="small", bufs=4))
            psum = ctx.enter_context(tc.tile_pool(name="psum", bufs=4, space="PSUM"))
            psum_r = ctx.enter_context(
                tc.tile_pool(name="psum_r", bufs=2, space="PSUM")
            )

            # ---- persistent tensors (fp16 slabs are [P, NT, n] = 16KB/part) ----
            x_hi = persist.tile([P, NT, n], F16, tag="x_hi")
            x_lo = persist.tile([P, NT, n], F16, tag="slotA")  # reused for w_hi
            # transposed slabs: [p, chunk, kt, j] = src-tile-major (contiguous
            # per chunk, required: non-contig xbar-transpose dst is broken)
            xT_hi = persist.tile([P, NT, NT, P], F16, tag="xT_hi")
            xT_lo = persist.tile([P, NT, NT, P], F16, tag="xT_lo")
            y_hi = persist.tile([P, NT, n], F16, tag="y_hi")
            y_lo = persist.tile([P, NT, n], F16, tag="slotD")  # reused for w_lo
            gt_hi = persist.tile([P, NT, n], F16, tag="gt_hi")
            gt_lo = persist.tile([P, NT, n], F16, tag="gt_lo")

            ones = persist.tile([P, 1], F16, tag="ones")
            nc.vector.memset(ones, 1.0)
            b_sb = persist.tile([1, n], F32, tag="b_sb")
            nc.sync.dma_start(b_sb, b_d[:, :])
            # scale bias so psum-group contribution is SLOG * rsx * b
            # (rsx lhsT carries SX): b_sc = b * SLOG/SX
            nc.vector.tensor_scalar_mul(b_sb, b_sb, SLOG / SX)
            rsx_sb = persist.tile([1, n], F32, tag="rsx_sb")
            recip = [
                persist.tile([P, 1], F32, tag=f"recip{i}", name=f"recip{i}")
                for i in range(NT)
            ]
            et = [
                persist.tile([P, NT, P], F16, tag=f"et{i}", name=f"et{i}")
                for i in range(NT)
            ]

            # ---- stage 0: load x, split, transpose halves -------------------
            for it in range(NT):
                xt = ld.tile([P, n], F32, tag="ld")
                nc.sync.dma_start(xt, x_d[P * it : P * (it + 1), :])
                nc.vector.tensor_scalar_mul(x_hi[:, it, :], xt, SX)
                nc.vector.scalar_tensor_tensor(
                    out=x_lo[:, it, :],
                    in0=xt,
                    scalar=SX,
                    in1=x_hi[:, it, :],
                    op0=ALU.mult,
                    op1=ALU.subtract,
                )
                nc.scalar.dma_start_transpose(xT_hi[:, it, :, :], x_hi[:, it, :])
                nc.scalar.dma_start_transpose(xT_lo[:, it, :, :], x_lo[:, it, :])

            # ---- load y, split ----------------------------------------------
            for kt in range(NT):
                yt = ld.tile([P, n], F32, tag="ld")
                nc.sync.dma_start(yt, y_d[P * kt : P * (kt + 1), :])
                nc.vector.tensor_scalar_mul(y_hi[:, kt, :], yt, SX)
                nc.vector.scalar_tensor_tensor(
                    out=y_lo[:, kt, :],
                    in0=yt,
                    scalar=SX,
                    in1=y_hi[:, kt, :],
                    op0=ALU.mult,
                    op1=ALU.subtract,
                )

            # ---- rsx[s] = SX * sum_k x[s,k]  (layout [1, n], for bias lhsT) --
            for h in range(NH):
                ps = psum_r.tile([1, FD], F32, tag="rsx")
                idx = 0
                for part in (xT_hi, xT_lo):
                    for kt in range(NT):
                        nc.tensor.matmul(
                            ps,
                            lhsT=ones,
                            rhs=part[:, 4 * h : 4 * h + 4, kt, :],
                            start=(idx == 0),
                            stop=(idx == 2 * NT - 1),
                        )
                        idx += 1
                nc.vector.tensor_copy(rsx_sb[0:1, FD * h : FD * (h + 1)], ps)

            # ---- g stage: gT[d, s] = SX^2 * sum_k x[s,k] y[k,d] -------------
            for dt in range(NT):
                for sh in range(NH):
                    ps = psum.tile([P, FD], F32, tag="mm")
                    idx = 0
                    for lhs, rhs in (
                        (y_hi, xT_hi),
                        (y_lo, xT_hi),
                        (y_hi, xT_lo),
                    ):
                        for kt in range(NT):
                            nc.tensor.matmul(
                                ps,
                                lhsT=lhs[:, kt, P * dt : P * (dt + 1)],
                                rhs=rhs[:, 4 * sh : 4 * sh + 4, kt, :],
                                start=(idx == 0),
                                stop=(idx == 3 * NT - 1),
                            )
                            idx += 1
                    nc.vector.tensor_copy(
                        gt_hi[:, dt, FD * sh : FD * (sh + 1)], ps
                    )
                    nc.vector.scalar_tensor_tensor(
                        out=gt_lo[:, dt, FD * sh : FD * (sh + 1)],
                        in0=ps,
                        scalar=1.0,
                        in1=gt_hi[:, dt, FD * sh : FD * (sh + 1)],
                        op0=ALU.mult,
                        op1=ALU.subtract,
                    )

            # ---- load W, split (reuses x_lo / y_lo slots) -------------------
            w_hi = persist.tile([P, NT, n], F16, tag="slotA")
            w_lo = persist.tile([P, NT, n], F16, tag="slotD")
            for dt in range(NT):
                wt = ld.tile([P, n], F32, tag="ld")
                nc.sync.dma_start(wt, w_d[P * dt : P * (dt + 1), :])
                nc.vector.tensor_scalar_mul(w_hi[:, dt, :], wt, SW)
                nc.vector.scalar_tensor_tensor(
                    out=w_lo[:, dt, :],
                    in0=wt,
                    scalar=SW,
                    in1=w_hi[:, dt, :],
                    op0=ALU.mult,
                    op1=ALU.subtract,
                )

            # ---- a stage + softmax ------------------------------------------
            for st in range(NT):
                mk = ld.tile([P, n], F32, tag="ld")
                nc.sync.dma_start(mk, mask_d[P * st : P * (st + 1), :])
                am = epi.tile([P, n], F32, tag="am")
                nc.vector.tensor_scalar_mul(am, mk, MASKC)
                for th in range(NH):
                    ps = psum.tile([P, FD], F32, tag="mm")
                    # rank-1 bias first (fp32): SLOG * rsx (x) b
                    nc.tensor.matmul(
                        ps,
                        lhsT=rsx_sb[0:1, P * st : P * (st + 1)],
                        rhs=b_sb[0:1, FD * th : FD * (th + 1)],
                        start=True,
                        stop=False,
                    )
                    idx = 0
                    for lhs, rhs in (
                        (gt_hi, w_hi),
                        (gt_lo, w_hi),
                        (gt_hi, w_lo),
                    ):
                        for dt in range(NT):
                            nc.tensor.matmul(
                                ps,
                                lhsT=lhs[:, dt, P * st : P * (st + 1)],
                                rhs=rhs[:, dt, FD * th : FD * (th + 1)],
                                start=False,
                                stop=(idx == 3 * NT - 1),
                            )
                            idx += 1
                    # masked scaled logits: am += psum (am pre-filled with mask*MASKC)
                    nc.vector.tensor_add(
                        out=am[:, FD * th : FD * (th + 1)],
                        in0=am[:, FD * th : FD * (th + 1)],
                        in1=ps,
                    )
                nm = small.tile([P, 1], F32, tag="nm")
                nc.vector.tensor_reduce(
                    nm, am, axis=AXIS.X, op=ALU.max, negate=True
                )
                nms = small.tile([P, 1], F32, tag="nms")
                nc.vector.tensor_scalar_mul(nms, nm, 1.0 / SLOG)
                eh = epi.tile([P, n], F16, tag="eh")
                rs = small.tile([P, 1], F32, tag="rs")
                nc.scalar.activation(
                    eh, am, ACTF.Exp, bias=nms, scale=1.0 / SLOG, accum_out=rs
                )
                nc.vector.reciprocal(recip[st], rs)
                nc.scalar.dma_start_transpose(et[st][:, :, :], eh)

            # ---- out stage: out[s, e] = (e_hat @ x_hi) * recip / SX ---------
            for st in range(NT):
                for h in range(NH):
                    ps = psum.tile([P, FD], F32, tag="mm")
                    for tt in range(NT):
                        nc.tensor.matmul(
                            ps,
                            lhsT=et[st][:, tt, :],
                            rhs=x_hi[:, tt, FD * h : FD * (h + 1)],
                            start=(tt == 0),
                            stop=(tt == NT - 1),
                        )
                    ob = epi.tile([P, FD], F32, tag="ob")
                    nc.vector.tensor_scalar(
                        ob,
                        ps,
                        recip[st],
                        1.0 / SX,
                        ALU.mult,
                        ALU.mult,
                    )
                    nc.sync.dma_start(
                        out_d[P * st : P * (st + 1), FD * h : FD * (h + 1)], ob
                    )
    nc.compile()
    return nc


_NC_CACHE = {}


def _get_nc(n=1024):
    if n not in _NC_CACHE:
        _NC_CACHE[n] = build_nc(n)
    return _NC_CACHE[n]


def kernel(x, y, mask, W, b):
    """Full-input entry point: shard over batch across 8 cores, run, gather."""
    n = x.shape[-1]
    nc = _get_nc(n)
    Wc = np.ascontiguousarray(W, dtype=np.float32)
    bc = np.ascontiguousarray(np.asarray(b, dtype=np.float32).reshape(1, n))
    in_maps = []
    for c in range(x.shape[0]):
        in_maps.append(
            {
                "x": np.ascontiguousarray(x[c], dtype=np.float32),
                "y": np.ascontiguousarray(y[c], dtype=np.float32),
                "mask": np.ascontiguousarray(mask[c], dtype=np.float32),
                "W": Wc,
                "bvec": bc,
            }
        )
    res = run_bass_kernel_spmd(nc, in_maps, core_ids=list(range(len(in_maps))))
    return np.stack([r["out"] for r in res.results], axis=0)


# revision 5
# speedup vs baseline: 1.1660x; 1.1660x over previous
# Trainium2 Bass kernel for nn_Attention: out = softmax(x @ (y@W + b) + mask*-1e9) @ x
# Sharding: data-parallel over batch, 1 batch element per NeuronCore (8 cores).
#
# Per-core math (S = D = 1024):
#   logits = x @ (y @ W) + rowsum(x) (x) b + mask * -1e9      [reassociated: (x@y)@W]
#   out    = softmax(logits) @ x
#
# Precision strategy: the logit chain runs as hi/lo-split fp16 matmuls (3 passes
# per matmul, fp16 products are exact in fp32 PSUM), giving near-fp32 logits at
# bf16-tier speed. The output matmul runs as a single fp16 pass. Inputs are
# pre-scaled (x,y by 16, W by 1024) so the fp16 "lo" residuals stay in normal
# fp16 range; the resulting 2^18 logit scale is folded into the exp affine.
#
# x is transposed on the tensor engine per 128x128 chunk (fine-grained deps so
# the PE ramps up immediately); the softmax output is transposed via DMA xbar.
import sys

import numpy as np

for _p in ("/opt/trn_rl_repo",):
    if _p not in sys.path:
        sys.path.insert(0, _p)

import concourse.bass as bass
from concourse import bacc
import concourse.mybir as mybir
import concourse.tile as tile
from concourse.bass_utils import run_bass_kernel_spmd
from concourse.masks import make_identity

F32 = mybir.dt.float32
F16 = mybir.dt.float16

B = 8
P = 128
FD = 512  # matmul moving free dim (one fp32 PSUM bank)

SX = 16.0  # x / y pre-scale
SW = 1024.0  # W pre-scale
SLOG = SX * SX * SW  # net logit scale = 2**18
MASKC = -1.0e9 * SLOG

ALU = mybir.AluOpType
ACTF = mybir.ActivationFunctionType
AXIS = mybir.AxisListType


def build_nc(n=1024):
    """Build the per-core Bass program (SPMD: same program on all 8 cores)."""
    NT = n // P  # 128-tiles per dim
    NH = n // FD  # 512-halves per dim
    HC = NT // NH  # transposed chunks per half (4)

    nc = bacc.Bacc("TRN2", target_bir_lowering=False, debug=False)
    x_d = nc.dram_tensor("x", [n, n], F32, kind="ExternalInput")
    y_d = nc.dram_tensor("y", [n, n], F32, kind="ExternalInput")
    mask_d = nc.dram_tensor("mask", [n, n], F32, kind="ExternalInput")
    w_d = nc.dram_tensor("W", [n, n], F32, kind="ExternalInput")
    b_d = nc.dram_tensor("bvec", [1, n], F32, kind="ExternalInput")
    out_d = nc.dram_tensor("out", [n, n], F32, kind="ExternalOutput")

    with tile.TileContext(nc) as tc:
        import contextlib

        ctx = contextlib.ExitStack()
        with ctx:
            persist = ctx.enter_context(tc.tile_pool(name="persist", bufs=1))
            ld = ctx.enter_context(tc.tile_pool(name="ld", bufs=3))
            epi = ctx.enter_context(tc.tile_pool(name="epi", bufs=2))
            small = ctx.enter_context(tc.tile_pool(name="small", bufs=4))
            psum = ctx.enter_context(tc.tile_pool(name="psum", bufs=4, space="PSUM"))
            psum_r = ctx.enter_context(
                tc.tile_pool(name="psum_r", bufs=1, space="PSUM")
            )
            psum_t = ctx.enter_context(
                tc.tile_pool(name="psum_t", bufs=3, space="PSUM")
            )

            # ---- persistent tensors (fp16 slabs are [P, NT, n] = 16KB/part) --
            x_hi = persist.tile([P, NT, n], F16, tag="x_hi")
            x_lo = persist.tile([P, NT, n], F16, tag="slotA")  # reused for w_hi
            y_hi = persist.tile([P, NT, n], F16, tag="y_hi")
            y_lo = persist.tile([P, NT, n], F16, tag="slotD")  # reused for w_lo
            gt_hi = persist.tile([P, NT, n], F16, tag="gt_hi")
            gt_lo = persist.tile([P, NT, n], F16, tag="gt_lo")
            # transposed x, one tile per (k-tile, s-half): [P, HC, P]
            xTh = [
                [
                    persist.tile(
                        [P, HC, P], F16, tag=f"xTh_{kt}_{h}", name=f"xTh_{kt}_{h}"
                    )
                    for h in range(NH)
                ]
                for kt in range(NT)
            ]
            xTl = [
                [
                    persist.tile(
                        [P, HC, P], F16, tag=f"xTl_{kt}_{h}", name=f"xTl_{kt}_{h}"
                    )
                    for h in range(NH)
                ]
                for kt in range(NT)
            ]

            ident = persist.tile([P, P], F16, tag="ident")
            make_identity(nc, ident)
            ones = persist.tile([P, 1], F16, tag="ones")
            nc.vector.memset(ones, 1.0)
            b_sb = persist.tile([1, n], F32, tag="b_sb")
            nc.sync.dma_start(b_sb, b_d[:, :])
            # scale bias so psum-group contribution is SLOG * rsx * b
            # (rsx lhsT carries SX): b_sc = b * SLOG/SX
            nc.vector.tensor_scalar_mul(b_sb, b_sb, SLOG / SX)
            rsx_sb = persist.tile([1, n], F32, tag="rsx_sb")
            recip = [
                persist.tile([P, 1], F32, tag=f"recip{i}", name=f"recip{i}")
                for i in range(NT)
            ]
            et = [
                persist.tile([P, NT, P], F16, tag=f"et{i}", name=f"et{i}")
                for i in range(NT)
            ]

            # ---- stage 0: load x, split, transpose chunks on PE -------------
            for it in range(NT):
                xt = ld.tile([P, n], F32, tag="ld")
                nc.sync.dma_start(xt, x_d[P * it : P * (it + 1), :])
                nc.vector.tensor_scalar_mul(x_hi[:, it, :], xt, SX)
                nc.vector.scalar_tensor_tensor(
                    out=x_lo[:, it, :],
                    in0=xt,
                    scalar=SX,
                    in1=x_hi[:, it, :],
                    op0=ALU.mult,
                    op1=ALU.subtract,
                )
                for c in range(NT):
                    pt = psum_t.tile([P, P], F16, tag="pt", name=f"pt_{it}_{c}h")
                    nc.tensor.transpose(pt, x_hi[:, it, P * c : P * (c + 1)], ident)
                    nc.scalar.copy(xTh[c][it // HC][:, it % HC, :], pt)
                    pl = psum_t.tile([P, P], F16, tag="pt", name=f"pt_{it}_{c}l")
                    nc.tensor.transpose(pl, x_lo[:, it, P * c : P * (c + 1)], ident)
                    nc.vector.tensor_copy(xTl[c][it // HC][:, it % HC, :], pl)

            # ---- load y (SWDGE queue, concurrent with x on HWDGE), split ----
            for kt in range(NT):
                yt = ld.tile([P, n], F32, tag="ld")
                nc.gpsimd.dma_start(yt, y_d[P * kt : P * (kt + 1), :])
                nc.vector.tensor_scalar_mul(y_hi[:, kt, :], yt, SX)
                nc.vector.scalar_tensor_tensor(
                    out=y_lo[:, kt, :],
                    in0=yt,
                    scalar=SX,
                    in1=y_hi[:, kt, :],
                    op0=ALU.mult,
                    op1=ALU.subtract,
                )

            # ---- g stage: gT[d, s] = SX^2 * sum_k x[s,k] y[k,d] -------------
            # sh-major so the first half only needs x tiles 0..HC-1 transposed.
            for sh in range(NH):
                for dt in range(NT):
                    ps = psum.tile([P, FD], F32, tag="mm")
                    idx = 0
                    for lhs, rhs in (
                        (y_hi, xTh),
                        (y_lo, xTh),
                        (y_hi, xTl),
                    ):
                        for kt in range(NT):
                            nc.tensor.matmul(
                                ps,
                                lhsT=lhs[:, kt, P * dt : P * (dt + 1)],
                                rhs=rhs[kt][sh][:, :, :],
                                start=(idx == 0),
                                stop=(idx == 3 * NT - 1),
                            )
                            idx += 1
                    nc.vector.tensor_copy(
                        gt_hi[:, dt, FD * sh : FD * (sh + 1)], ps
                    )
                    nc.vector.scalar_tensor_tensor(
                        out=gt_lo[:, dt, FD * sh : FD * (sh + 1)],
                        in0=ps,
                        scalar=1.0,
                        in1=gt_hi[:, dt, FD * sh : FD * (sh + 1)],
                        op0=ALU.mult,
                        op1=ALU.subtract,
                    )

            # ---- rsx[s] = SX * sum_k x[s,k]  (layout [1, n], for bias lhsT) --
            for h in range(NH):
                ps = psum_r.tile([1, FD], F32, tag="rsx")
                idx = 0
                for part in (xTh, xTl):
                    for kt in range(NT):
                        nc.tensor.matmul(
                            ps,
                            lhsT=ones,
                            rhs=part[kt][h][:, :, :],
                            start=(idx == 0),
                            stop=(idx == 2 * NT - 1),
                        )
                        idx += 1
                nc.vector.tensor_copy(rsx_sb[0:1, FD * h : FD * (h + 1)], ps)

            # ---- load W (SWDGE), split (reuses x_lo / y_lo slots) -----------
            w_hi = persist.tile([P, NT, n], F16, tag="slotA")
            w_lo = persist.tile([P, NT, n], F16, tag="slotD")
            for dt in range(NT):
                wt = ld.tile([P, n], F32, tag="ld")
                nc.gpsimd.dma_start(wt, w_d[P * dt : P * (dt + 1), :])
                nc.vector.tensor_scalar_mul(w_hi[:, dt, :], wt, SW)
                nc.vector.scalar_tensor_tensor(
                    out=w_lo[:, dt, :],
                    in0=wt,
                    scalar=SW,
                    in1=w_hi[:, dt, :],
                    op0=ALU.mult,
                    op1=ALU.subtract,
                )

            # ---- a stage + softmax ------------------------------------------
            for st in range(NT):
                mk = ld.tile([P, n], F32, tag="ld")
                nc.sync.dma_start(mk, mask_d[P * st : P * (st + 1), :])
                am = epi.tile([P, n], F32, tag="am")
                nc.vector.tensor_scalar_mul(am, mk, MASKC)
                for th in range(NH):
                    ps = psum.tile([P, FD], F32, tag="mm")
                    # rank-1 bias first (fp32): SLOG * rsx (x) b
                    nc.tensor.matmul(
                        ps,
                        lhsT=rsx_sb[0:1, P * st : P * (st + 1)],
                        rhs=b_sb[0:1, FD * th : FD * (th + 1)],
                        start=True,
                        stop=False,
                    )
                    idx = 0
                    for lhs, rhs in (
                        (gt_hi, w_hi),
                        (gt_lo, w_hi),
                        (gt_hi, w_lo),
                    ):
                        for dt in range(NT):
                            nc.tensor.matmul(
                                ps,
                                lhsT=lhs[:, dt, P * st : P * (st + 1)],
                                rhs=rhs[:, dt, FD * th : FD * (th + 1)],
                                start=False,
                                stop=(idx == 3 * NT - 1),
                            )
                            idx += 1
                    # masked scaled logits: am += psum (am pre-set to mask*MASKC)
                    nc.vector.tensor_add(
                        out=am[:, FD * th : FD * (th + 1)],
                        in0=am[:, FD * th : FD * (th + 1)],
                        in1=ps,
                    )
                nm = small.tile([P, 1], F32, tag="nm")
                nc.vector.tensor_reduce(
                    nm, am, axis=AXIS.X, op=ALU.max, negate=True
                )
                nms = small.tile([P, 1], F32, tag="nms")
                nc.vector.tensor_scalar_mul(nms, nm, 1.0 / SLOG)
                eh = epi.tile([P, n], F16, tag="eh")
                rs = small.tile([P, 1], F32, tag="rs")
                nc.scalar.activation(
                    eh, am, ACTF.Exp, bias=nms, scale=1.0 / SLOG, accum_out=rs
                )
                nc.vector.reciprocal(recip[st], rs)
                nc.scalar.dma_start_transpose(et[st][:, :, :], eh)

            # ---- out stage: out[s, e] = (e_hat @ x_hi) * recip / SX ---------
            for st in range(NT):
                for h in range(NH):
                    ps = psum.tile([P, FD], F32, tag="mm")
                    for tt in range(NT):
                        nc.tensor.matmul(
                            ps,
                            lhsT=et[st][:, tt, :],
                            rhs=x_hi[:, tt, FD * h : FD * (h + 1)],
                            start=(tt == 0),
                            stop=(tt == NT - 1),
                        )
                    ob = epi.tile([P, FD], F32, tag="ob")
                    nc.vector.tensor_scalar(
                        ob,
                        ps,
                        recip[st],
                        1.0 / SX,
                        ALU.mult,
                        ALU.mult,
                    )
                    nc.sync.dma_start(
                        out_d[P * st : P * (st + 1), FD * h : FD * (h + 1)], ob
                    )
    nc.compile()
    return nc


_NC_CACHE = {}


def _get_nc(n=1024):
    if n not in _NC_CACHE:
        _NC_CACHE[n] = build_nc(n)
    return _NC_CACHE[n]


def kernel(x, y, mask, W, b):
    """Full-input entry point: shard over batch across 8 cores, run, gather."""
    n = x.shape[-1]
    nc = _get_nc(n)
    Wc = np.ascontiguousarray(W, dtype=np.float32)
    bc = np.ascontiguousarray(np.asarray(b, dtype=np.float32).reshape(1, n))
    in_maps = []
    for c in range(x.shape[0]):
        in_maps.append(
            {
                "x": np.ascontiguousarray(x[c], dtype=np.float32),
                "y": np.ascontiguousarray(y[c], dtype=np.float32),
                "mask": np.ascontiguousarray(mask[c], dtype=np.float32),
                "W": Wc,
                "bvec": bc,
            }
        )
    res = run_bass_kernel_spmd(nc, in_maps, core_ids=list(range(len(in_maps))))
    return np.stack([r["out"] for r in res.results], axis=0)
